# revision 18
# baseline (speedup 1.0000x reference)
"""Chamfer distance kernel for 8 Trainium2 NeuronCores.

Problem: pred/target (4, 8192, 3) fp32 -> scalar mean chamfer distance
(bidirectional nearest-neighbor squared distances, mean over batch).

Sharding (data parallel on batch x pred-half): core c handles batch
b = c // 2 and pred-half h = c % 2 (4096 of the 8192 pred points) against
ALL 8192 targets of that batch. Forward mins (over targets) complete per
core; backward row-mins (over preds) are per-half partials that the host
min-combines across the core pair.

Device math per core:
  d2[m, n] = ||q_m||^2 + ||p_n||^2 - 2 q_m . p_n   (m target, n pred)
computed as one K=13 matmul using bf16 hi/lo splitting (~2^-18 relative
error, and bf16 streams at 1 PE cycle/row where fp32 needs 4):
    Q_aug rows: [qh0..2, qh0..2, ql0..2, q2h, q2l, 1, 1]  (q* = split(-2q))
    P_aug rows: [ph0..2, pl0..2, ph0..2, 1,  1,  p2h, p2l]
  dot = qh.ph + qh.pl + ql.ph + q2 + p2 ~= -2 q.p + ||q||^2 + ||p||^2.

Tiling: out[m-chunk of 128 on partitions, n-chunk of 512 free] in PSUM;
4 banks (2048 free) per reduction group, double buffered (8 banks total).
Per group, two DVE instructions consume the fresh d2 values:
  1) tensor_reduce(min, free axis) on psum alone -> r[:, g*64+mi]
     (clean backward row-min for these 128 targets over this n-group)
  2) tensor_tensor_reduce: A[:, g] = min(psum, A[:, g]) elementwise
     (forward min accumulate; the fused reduce output is a running min
     contaminated by A and is discarded into a scratch slot).
Clamping to zero (reference's maximum(d2, 0)) commutes with min, so it is
applied on the host after all mins.

Outputs per core: A [128, 4096] fp32 (forward; partition-min left to the
host) and r [128, 128] fp32 (backward row-mins per (g, mi)). The host does
the partition mins, relu clamp, cross-core/pair combines and means in
float64, returning the fp32 scalar.
"""

import functools

import numpy as np
import ml_dtypes

import concourse.bacc as bacc
import concourse.mybir as mybir
import concourse.tile as tile
from concourse.bass_utils import run_bass_kernel_spmd

BF16 = ml_dtypes.bfloat16

B = 4            # batches
N = 8192         # points per cloud
NCORES = 8
NH = N // 2      # preds per core (4096)
K = 13           # augmented contraction dim
MI = N // 128    # 64 target chunks of 128
GF = 2048        # free elements per reduction group (4 psum banks)
NG = NH // GF    # 2 groups per mi
BIG = 3.0e38


def _split_bf16(x):
    """fp32 -> (hi, lo) bf16 pair with x ~= hi + lo (error ~2^-18 |x|)."""
    xh = x.astype(BF16)
    xl = (x - xh.astype(np.float32)).astype(BF16)
    return xh, xl


def _aug_inputs(pred, target):
    """Per-core augmented bf16 matrices: {"q_aug": [13, 8192], "p_aug": [13, 4096]}."""
    in_maps = []
    for c in range(NCORES):
        b, h = divmod(c, 2)
        q = np.asarray(target[b], dtype=np.float32)              # (8192, 3)
        p = np.asarray(pred[b][h * NH:(h + 1) * NH], dtype=np.float32)

        qh, ql = _split_bf16(-2.0 * q)
        q2h, q2l = _split_bf16(np.sum(q * q, axis=-1, dtype=np.float32))
        onesq = np.ones(N, dtype=BF16)
        q_aug = np.stack([
            qh[:, 0], qh[:, 1], qh[:, 2],
            qh[:, 0], qh[:, 1], qh[:, 2],
            ql[:, 0], ql[:, 1], ql[:, 2],
            q2h, q2l, onesq, onesq,
        ])                                                       # (13, 8192)

        ph, pl = _split_bf16(p)
        p2h, p2l = _split_bf16(np.sum(p * p, axis=-1, dtype=np.float32))
        onesp = np.ones(NH, dtype=BF16)
        p_aug = np.stack([
            ph[:, 0], ph[:, 1], ph[:, 2],
            pl[:, 0], pl[:, 1], pl[:, 2],
            ph[:, 0], ph[:, 1], ph[:, 2],
            onesp, onesp, p2h, p2l,
        ])                                                       # (13, 4096)
        in_maps.append({"q_aug": np.ascontiguousarray(q_aug),
                        "p_aug": np.ascontiguousarray(p_aug)})
    return in_maps


# Forward-accumulate split point within each 2048-wide group (columns
# [0, SPLIT) go to the DVE, the rest to GPSIMD) — see mode "stage" below.
SPLIT = 256


@functools.lru_cache(maxsize=8)
def _build_program(mi_count=MI, split=SPLIT, mode="bf16fold"):
    """mode:
    "alldve":  DVE does fp32 reduce + full min-accumulate straight from PSUM.
    "bf16fold": ACT is the sole PSUM reader, casting each group to bf16 in
               SBUF. DVE then runs at 2x (bf16 tensor_tensor): folds the two
               n-groups of an mi (valid for the backward row-min), reduces
               the fold once, and min-accumulates both groups into a bf16 A.
               bf16 rounding adds ~1e-4 relative error to the final scalar.
    """
    nc = bacc.Bacc("TRN2", target_bir_lowering=False, debug=False,
                   num_devices=NCORES)
    f32 = mybir.dt.float32
    bf16 = mybir.dt.bfloat16
    mn = mybir.AluOpType.min

    a_dt = f32 if mode == "alldve" else bf16
    r_cols = NG * MI if mode == "alldve" else MI

    q_dram = nc.dram_tensor("q_aug", [K, N], bf16, kind="ExternalInput")
    p_dram = nc.dram_tensor("p_aug", [K, NH], bf16, kind="ExternalInput")
    a_dram = nc.dram_tensor("a_out", [128, NH], a_dt, kind="ExternalOutput")
    r_dram = nc.dram_tensor("r_out", [128, r_cols], f32, kind="ExternalOutput")

    with tile.TileContext(nc) as tc:
        with tc.tile_pool(name="const", bufs=1) as cpool, \
             tc.tile_pool(name="stage", bufs=4) as spool, \
             tc.tile_pool(name="fold", bufs=2) as fpool, \
             tc.tile_pool(name="psum", bufs=2, space="PSUM") as ppool:
            q_sb = cpool.tile([K, N], bf16)
            p_sb = cpool.tile([K, NH], bf16)
            a_sb = cpool.tile([128, NH], a_dt)
            r_sb = cpool.tile([128, r_cols], f32)

            nc.sync.dma_start(out=q_sb[:], in_=q_dram.ap())
            nc.sync.dma_start(out=p_sb[:], in_=p_dram.ap())
            nc.gpsimd.memset(a_sb[:], BIG)

            for mi in range(mi_count):
                stg = []
                for g in range(NG):
                    ps = ppool.tile([128, GF], f32, tag="ps")
                    for j in range(GF // 512):
                        nj = (GF // 512) * g + j
                        nc.tensor.matmul(
                            ps[:, j * 512:(j + 1) * 512],
                            q_sb[:, mi * 128:(mi + 1) * 128],
                            p_sb[:, nj * 512:(nj + 1) * 512],
                            start=True, stop=True,
                        )
                    asl = a_sb[:, g * GF:(g + 1) * GF]
                    if mode == "alldve":
                        nc.vector.tensor_reduce(
                            out=r_sb[:, g * MI + mi: g * MI + mi + 1],
                            in_=ps[:], axis=mybir.AxisListType.X, op=mn)
                        nc.vector.tensor_tensor(
                            out=asl, in0=ps[:], in1=asl, op=mn)
                    elif mode == "bf16fold":
                        s_sb = spool.tile([128, GF], bf16, tag="stage")
                        nc.scalar.copy(s_sb[:], ps[:])
                        nc.vector.tensor_tensor(
                            out=asl, in0=s_sb[:], in1=asl, op=mn)
                        stg.append(s_sb)
                    else:
                        raise ValueError(mode)
                if mode == "bf16fold":
                    fold = fpool.tile([128, GF], bf16, tag="fold")
                    nc.vector.tensor_tensor(
                        out=fold[:], in0=stg[0][:], in1=stg[1][:], op=mn)
                    nc.vector.tensor_reduce(
                        out=r_sb[:, mi: mi + 1], in_=fold[:],
                        axis=mybir.AxisListType.X, op=mn)

            nc.sync.dma_start(out=a_dram.ap(), in_=a_sb[:])
            nc.sync.dma_start(out=r_dram.ap(), in_=r_sb[:])

    nc.compile()
    return nc


def _host_reduce(results):
    """Combine per-core outputs into the final scalar (float64 internally)."""
    chamfers = []
    for b in range(B):
        fs = []
        bvecs = []
        for h in range(2):
            res = results[2 * b + h]
            A = np.asarray(res["a_out"]).astype(np.float64)   # [128, 4096]
            R = np.asarray(res["r_out"]).astype(np.float64)   # [128, 64|128]
            fs.append(A.min(axis=0))                          # [4096]
            if R.shape[1] == 2 * MI:                          # per-g columns
                R = np.minimum(R[:, :MI], R[:, MI:])          # [128, 64]
            bvecs.append(R.T.reshape(N))                      # m = 128*mi + p
        f = np.maximum(np.concatenate(fs), 0.0)               # [8192] fwd mins
        bv = np.maximum(np.minimum(bvecs[0], bvecs[1]), 0.0)  # [8192] bwd mins
        chamfers.append(f.mean() + bv.mean())
    return np.float32(np.mean(chamfers))


def kernel(pred, target):
    pred = np.asarray(pred, dtype=np.float32)
    target = np.asarray(target, dtype=np.float32)
    in_maps = _aug_inputs(pred, target)
    nc = _build_program()
    res = run_bass_kernel_spmd(nc, in_maps, core_ids=list(range(NCORES)))
    return np.array(_host_reduce(res.results), dtype=np.float32)


# revision 20
# speedup vs baseline: 1.1194x; 1.1194x over previous
"""Chamfer distance kernel for 8 Trainium2 NeuronCores.

Problem: pred/target (4, 8192, 3) fp32 -> scalar mean chamfer distance
(bidirectional nearest-neighbor squared distances, mean over batch).

Sharding (data parallel on batch x pred-half): core c handles batch
b = c // 2 and pred-half h = c % 2 (4096 of the 8192 pred points) against
ALL 8192 targets of that batch. Forward mins (over targets) complete per
core; backward row-mins (over preds) are per-half partials that the host
min-combines across the core pair.

Device math per core:
  d2[m, n] = ||q_m||^2 + ||p_n||^2 - 2 q_m . p_n   (m target, n pred)
computed as one K=13 matmul using bf16 hi/lo splitting (~2^-18 relative
error, and bf16 streams at 1 PE cycle/row where fp32 needs 4):
    Q_aug rows: [qh0..2, qh0..2, ql0..2, q2h, q2l, 1, 1]  (q* = split(-2q))
    P_aug rows: [ph0..2, pl0..2, ph0..2, 1,  1,  p2h, p2l]
  dot = qh.ph + qh.pl + ql.ph + q2 + p2 ~= -2 q.p + ||q||^2 + ||p||^2.

Tiling: out[m-chunk of 128 on partitions, n-chunk of 512 free] in PSUM;
4 banks (2048 free) per reduction group, double buffered (8 banks total).
Per group, two DVE instructions consume the fresh d2 values:
  1) tensor_reduce(min, free axis) on psum alone -> r[:, g*64+mi]
     (clean backward row-min for these 128 targets over this n-group)
  2) tensor_tensor_reduce: A[:, g] = min(psum, A[:, g]) elementwise
     (forward min accumulate; the fused reduce output is a running min
     contaminated by A and is discarded into a scratch slot).
Clamping to zero (reference's maximum(d2, 0)) commutes with min, so it is
applied on the host after all mins.

Outputs per core: A [128, 4096] fp32 (forward; partition-min left to the
host) and r [128, 128] fp32 (backward row-mins per (g, mi)). The host does
the partition mins, relu clamp, cross-core/pair combines and means in
float64, returning the fp32 scalar.
"""

import functools

import numpy as np
import ml_dtypes

import concourse.bacc as bacc
import concourse.mybir as mybir
import concourse.tile as tile
from concourse.bass_utils import run_bass_kernel_spmd

BF16 = ml_dtypes.bfloat16

B = 4            # batches
N = 8192         # points per cloud
NCORES = 8
NH = N // 2      # preds per core (4096)
K = 13           # augmented contraction dim
MI = N // 128    # 64 target chunks of 128
GF = 2048        # free elements per reduction group (4 psum banks)
NG = NH // GF    # 2 groups per mi
BIG = 3.0e38


def _split_bf16(x):
    """fp32 -> (hi, lo) bf16 pair with x ~= hi + lo (error ~2^-18 |x|)."""
    xh = x.astype(BF16)
    xl = (x - xh.astype(np.float32)).astype(BF16)
    return xh, xl


def _aug_inputs(pred, target):
    """Per-core augmented bf16 matrices: {"q_aug": [13, 8192], "p_aug": [13, 4096]}."""
    in_maps = []
    for c in range(NCORES):
        b, h = divmod(c, 2)
        q = np.asarray(target[b], dtype=np.float32)              # (8192, 3)
        p = np.asarray(pred[b][h * NH:(h + 1) * NH], dtype=np.float32)

        qh, ql = _split_bf16(-2.0 * q)
        q2h, q2l = _split_bf16(np.sum(q * q, axis=-1, dtype=np.float32))
        onesq = np.ones(N, dtype=BF16)
        q_aug = np.stack([
            qh[:, 0], qh[:, 1], qh[:, 2],
            qh[:, 0], qh[:, 1], qh[:, 2],
            ql[:, 0], ql[:, 1], ql[:, 2],
            q2h, q2l, onesq, onesq,
        ])                                                       # (13, 8192)

        ph, pl = _split_bf16(p)
        p2h, p2l = _split_bf16(np.sum(p * p, axis=-1, dtype=np.float32))
        onesp = np.ones(NH, dtype=BF16)
        p_aug = np.stack([
            ph[:, 0], ph[:, 1], ph[:, 2],
            pl[:, 0], pl[:, 1], pl[:, 2],
            ph[:, 0], ph[:, 1], ph[:, 2],
            onesp, onesp, p2h, p2l,
        ])                                                       # (13, 4096)
        in_maps.append({"q_aug": np.ascontiguousarray(q_aug),
                        "p_aug": np.ascontiguousarray(p_aug)})
    return in_maps


# Forward-accumulate split point within each 2048-wide group (columns
# [0, SPLIT) go to the DVE, the rest to GPSIMD) — see mode "stage" below.
SPLIT = 256


@functools.lru_cache(maxsize=8)
def _build_program(mi_count=MI, split=SPLIT, mode="bf16fold"):
    """mode:
    "alldve":  DVE does fp32 reduce + full min-accumulate straight from PSUM.
    "bf16fold": ACT is the sole PSUM reader, casting each group to bf16 in
               SBUF. DVE then runs at 2x (bf16 tensor_tensor): folds the two
               n-groups of an mi (valid for the backward row-min), reduces
               the fold once, and min-accumulates both groups into a bf16 A.
               bf16 rounding adds ~1e-4 relative error to the final scalar.
    """
    nc = bacc.Bacc("TRN2", target_bir_lowering=False, debug=False,
                   num_devices=NCORES)
    f32 = mybir.dt.float32
    bf16 = mybir.dt.bfloat16
    mn = mybir.AluOpType.min

    a_dt = f32 if mode == "alldve" else bf16
    r_cols = NG * MI if mode == "alldve" else MI

    q_dram = nc.dram_tensor("q_aug", [K, N], bf16, kind="ExternalInput")
    p_dram = nc.dram_tensor("p_aug", [K, NH], bf16, kind="ExternalInput")
    a_dram = nc.dram_tensor("a_out", [128, NH], a_dt, kind="ExternalOutput")
    r_dram = nc.dram_tensor("r_out", [128, r_cols], f32, kind="ExternalOutput")

    with tile.TileContext(nc) as tc:
        with tc.tile_pool(name="const", bufs=1) as cpool, \
             tc.tile_pool(name="stage", bufs=4) as spool, \
             tc.tile_pool(name="fold", bufs=2) as fpool, \
             tc.tile_pool(name="psum", bufs=2, space="PSUM") as ppool:
            q_sb = cpool.tile([K, N], bf16)
            p_sb = cpool.tile([K, NH], bf16)
            a_sb = cpool.tile([128, NH], a_dt)
            r_sb = cpool.tile([128, r_cols], f32)

            nc.sync.dma_start(out=q_sb[:], in_=q_dram.ap())
            nc.sync.dma_start(out=p_sb[:], in_=p_dram.ap())
            nc.gpsimd.memset(a_sb[:], BIG)

            for mi in range(mi_count):
                stg = []
                for g in range(NG):
                    ps = ppool.tile([128, GF], f32, tag="ps")
                    for j in range(GF // 512):
                        nj = (GF // 512) * g + j
                        nc.tensor.matmul(
                            ps[:, j * 512:(j + 1) * 512],
                            q_sb[:, mi * 128:(mi + 1) * 128],
                            p_sb[:, nj * 512:(nj + 1) * 512],
                            start=True, stop=True,
                        )
                    if mode == "alldve":
                        asl = a_sb[:, g * GF:(g + 1) * GF]
                        nc.vector.tensor_reduce(
                            out=r_sb[:, g * MI + mi: g * MI + mi + 1],
                            in_=ps[:], axis=mybir.AxisListType.X, op=mn)
                        nc.vector.tensor_tensor(
                            out=asl, in0=ps[:], in1=asl, op=mn)
                    elif mode == "bf16fold":
                        if g == 0:
                            s_sb = spool.tile([128, NG * GF], bf16,
                                              tag="stage")
                            stg.append(s_sb)
                        nc.scalar.copy(stg[0][:, g * GF:(g + 1) * GF], ps[:])
                    else:
                        raise ValueError(mode)
                if mode == "bf16fold":
                    s_sb = stg[0]
                    # forward min-accumulate, both groups in one 2x bf16 op
                    nc.vector.tensor_tensor(
                        out=a_sb[:], in0=s_sb[:], in1=a_sb[:], op=mn)
                    # backward: fold the two n-groups (valid for the row-min),
                    # then min-halve at 2x before the 1x-rate reduce
                    fold = fpool.tile([128, GF], bf16, tag="fold")
                    nc.vector.tensor_tensor(
                        out=fold[:], in0=s_sb[:, :GF], in1=s_sb[:, GF:],
                        op=mn)
                    w = GF // 2
                    while w >= 256:
                        nc.vector.tensor_tensor(
                            out=fold[:, :w], in0=fold[:, :w],
                            in1=fold[:, w:2 * w], op=mn)
                        w //= 2
                    nc.vector.tensor_reduce(
                        out=r_sb[:, mi: mi + 1], in_=fold[:, :2 * w],
                        axis=mybir.AxisListType.X, op=mn)

            nc.sync.dma_start(out=a_dram.ap(), in_=a_sb[:])
            nc.sync.dma_start(out=r_dram.ap(), in_=r_sb[:])

    nc.compile()
    return nc


def _host_reduce(results):
    """Combine per-core outputs into the final scalar (float64 internally)."""
    chamfers = []
    for b in range(B):
        fs = []
        bvecs = []
        for h in range(2):
            res = results[2 * b + h]
            A = np.asarray(res["a_out"]).astype(np.float64)   # [128, 4096]
            R = np.asarray(res["r_out"]).astype(np.float64)   # [128, 64|128]
            fs.append(A.min(axis=0))                          # [4096]
            if R.shape[1] == 2 * MI:                          # per-g columns
                R = np.minimum(R[:, :MI], R[:, MI:])          # [128, 64]
            bvecs.append(R.T.reshape(N))                      # m = 128*mi + p
        f = np.maximum(np.concatenate(fs), 0.0)               # [8192] fwd mins
        bv = np.maximum(np.minimum(bvecs[0], bvecs[1]), 0.0)  # [8192] bwd mins
        chamfers.append(f.mean() + bv.mean())
    return np.float32(np.mean(chamfers))


def kernel(pred, target):
    pred = np.asarray(pred, dtype=np.float32)
    target = np.asarray(target, dtype=np.float32)
    in_maps = _aug_inputs(pred, target)
    nc = _build_program()
    res = run_bass_kernel_spmd(nc, in_maps, core_ids=list(range(NCORES)))
    return np.array(_host_reduce(res.results), dtype=np.float32)


# revision 21
# speedup vs baseline: 1787.4244x; 1596.8401x over previous
"""Chamfer distance kernel for 8 Trainium2 NeuronCores (Bass/Tile).

Problem: pred/target (4, 8192, 3) fp32 -> scalar mean chamfer distance
(bidirectional nearest-neighbor squared distances, mean over batch).

Sharding (data parallel on batch x pred-half): core c handles batch
b = c // 2 and pred-half h = c % 2 (4096 of the 8192 pred points) against
ALL 8192 targets of that batch. Forward mins (over targets) complete per
core; backward row-mins (over preds) are per-half partials that the host
min-combines across the core pair.

Device math per core:
  d2[m, n] = ||q_m||^2 + ||p_n||^2 - 2 q_m . p_n   (m target, n pred)
as ONE K=13 matmul per tile using bf16 hi/lo splitting (~2^-18 relative
error; bf16 streams 1 PE cycle/row where fp32 needs 4):
    Q_aug rows: [qh0..2, qh0..2, ql0..2, q2h, q2l, 1, 1]  (q* = split(-2q))
    P_aug rows: [ph0..2, pl0..2, ph0..2, 1,  1,  p2h, p2l]
  dot = qh.ph + qh.pl + ql.ph + q2 + p2 ~= -2 q.p + ||q||^2 + ||p||^2.

Pipeline per target chunk mi (128 targets on PSUM partitions, all 4096
preds on the free axis, 8 matmuls of [13,128]x[13,512] into two 4-bank
PSUM groups, double buffered):
  - ScalarE (sole PSUM reader -> no cross-engine PSUM bank serialization)
    copies each group into one contiguous [128, 4096] bf16 staging tile.
  - VectorE runs everything at the bf16 2x tensor_tensor rate:
      * forward: A = min(A, staged) in one [128, 4096] op (A bf16),
      * backward: fold the two n-groups (valid under the row-min), then
        min-halve 2048 -> 256 at 2x before the 1x-rate tensor_reduce that
        produces this mi's row-min column r[:, mi].
  The DVE touches each d2 element ~1.1x at 2 elem/cycle/lane, which is the
  bottleneck engine (~300us/core; TimelineSim ~318us).

The reference's maximum(d2, 0) clamp commutes with min, so the host
applies it after all mins. The host also does the final cross-partition
mins, cross-core combines and means in float64 (device values are exact
bf16/fp32 mins, so this adds no device work).

bf16 rounding of d2 before the min reductions adds ~1e-4 relative error
to the final scalar (validated ~1e-5 on both PRNG variants of the
reference inputs); the matmul's hi/lo split error is ~2^-18 per term.
"""

import functools

import numpy as np
import ml_dtypes

import concourse.bacc as bacc
import concourse.mybir as mybir
import concourse.tile as tile

BF16 = ml_dtypes.bfloat16

B = 4            # batches
N = 8192         # points per cloud
NCORES = 8
NH = N // 2      # preds per core (4096)
K = 13           # augmented contraction dim
MI = N // 128    # 64 target chunks of 128
GF = 2048        # free elements per psum group (4 banks)
NG = NH // GF    # 2 groups per mi
BIG = 3.0e38


def _split_bf16(x):
    """fp32 -> (hi, lo) bf16 pair with x ~= hi + lo (error ~2^-18 |x|)."""
    xh = x.astype(BF16)
    xl = (x - xh.astype(np.float32)).astype(BF16)
    return xh, xl


def _aug_inputs(pred, target):
    """Per-core augmented bf16 matrices: {"q_aug": [13, 8192], "p_aug": [13, 4096]}."""
    in_maps = []
    for c in range(NCORES):
        b, h = divmod(c, 2)
        q = np.asarray(target[b], dtype=np.float32)              # (8192, 3)
        p = np.asarray(pred[b][h * NH:(h + 1) * NH], dtype=np.float32)

        qh, ql = _split_bf16(-2.0 * q)
        q2h, q2l = _split_bf16(np.sum(q * q, axis=-1, dtype=np.float32))
        onesq = np.ones(N, dtype=BF16)
        q_aug = np.stack([
            qh[:, 0], qh[:, 1], qh[:, 2],
            qh[:, 0], qh[:, 1], qh[:, 2],
            ql[:, 0], ql[:, 1], ql[:, 2],
            q2h, q2l, onesq, onesq,
        ])                                                       # (13, 8192)

        ph, pl = _split_bf16(p)
        p2h, p2l = _split_bf16(np.sum(p * p, axis=-1, dtype=np.float32))
        onesp = np.ones(NH, dtype=BF16)
        p_aug = np.stack([
            ph[:, 0], ph[:, 1], ph[:, 2],
            pl[:, 0], pl[:, 1], pl[:, 2],
            ph[:, 0], ph[:, 1], ph[:, 2],
            onesp, onesp, p2h, p2l,
        ])                                                       # (13, 4096)
        in_maps.append({"q_aug": np.ascontiguousarray(q_aug),
                        "p_aug": np.ascontiguousarray(p_aug)})
    return in_maps


@functools.lru_cache(maxsize=4)
def _build_program(mi_count=MI, mode="bf16fold"):
    """mode "alldve": fp32 reduce + min-accumulate straight from PSUM (no
    bf16 rounding, ~2x slower). mode "bf16fold": the pipeline described in
    the module docstring."""
    nc = bacc.Bacc("TRN2", target_bir_lowering=False, debug=False,
                   num_devices=NCORES)
    f32 = mybir.dt.float32
    bf16 = mybir.dt.bfloat16
    mn = mybir.AluOpType.min

    a_dt = f32 if mode == "alldve" else bf16
    r_cols = NG * MI if mode == "alldve" else MI

    q_dram = nc.dram_tensor("q_aug", [K, N], bf16, kind="ExternalInput")
    p_dram = nc.dram_tensor("p_aug", [K, NH], bf16, kind="ExternalInput")
    a_dram = nc.dram_tensor("a_out", [128, NH], a_dt, kind="ExternalOutput")
    r_dram = nc.dram_tensor("r_out", [128, r_cols], f32, kind="ExternalOutput")

    with tile.TileContext(nc) as tc:
        with tc.tile_pool(name="const", bufs=1) as cpool, \
             tc.tile_pool(name="stage", bufs=3) as spool, \
             tc.tile_pool(name="fold", bufs=2) as fpool, \
             tc.tile_pool(name="psum", bufs=2, space="PSUM") as ppool:
            q_sb = cpool.tile([K, N], bf16)
            p_sb = cpool.tile([K, NH], bf16)
            a_sb = cpool.tile([128, NH], a_dt)
            r_sb = cpool.tile([128, r_cols], f32)

            nc.sync.dma_start(out=q_sb[:], in_=q_dram.ap())
            nc.sync.dma_start(out=p_sb[:], in_=p_dram.ap())
            nc.gpsimd.memset(a_sb[:], BIG)

            for mi in range(mi_count):
                s_sb = None
                for g in range(NG):
                    ps = ppool.tile([128, GF], f32, tag="ps")
                    for j in range(GF // 512):
                        nj = (GF // 512) * g + j
                        nc.tensor.matmul(
                            ps[:, j * 512:(j + 1) * 512],
                            q_sb[:, mi * 128:(mi + 1) * 128],
                            p_sb[:, nj * 512:(nj + 1) * 512],
                            start=True, stop=True,
                        )
                    if mode == "alldve":
                        asl = a_sb[:, g * GF:(g + 1) * GF]
                        nc.vector.tensor_reduce(
                            out=r_sb[:, g * MI + mi: g * MI + mi + 1],
                            in_=ps[:], axis=mybir.AxisListType.X, op=mn)
                        nc.vector.tensor_tensor(
                            out=asl, in0=ps[:], in1=asl, op=mn)
                    elif mode == "bf16fold":
                        if s_sb is None:
                            s_sb = spool.tile([128, NG * GF], bf16,
                                              tag="stage")
                        nc.scalar.copy(s_sb[:, g * GF:(g + 1) * GF], ps[:])
                    else:
                        raise ValueError(mode)
                if mode == "bf16fold":
                    # forward min-accumulate, both groups in one 2x bf16 op
                    nc.vector.tensor_tensor(
                        out=a_sb[:], in0=s_sb[:], in1=a_sb[:], op=mn)
                    # backward: fold the two n-groups (valid for the
                    # row-min), min-halve at 2x, then the 1x-rate reduce
                    fold = fpool.tile([128, GF], bf16, tag="fold")
                    nc.vector.tensor_tensor(
                        out=fold[:], in0=s_sb[:, :GF], in1=s_sb[:, GF:],
                        op=mn)
                    w = GF // 2
                    while w >= 256:
                        nc.vector.tensor_tensor(
                            out=fold[:, :w], in0=fold[:, :w],
                            in1=fold[:, w:2 * w], op=mn)
                        w //= 2
                    nc.vector.tensor_reduce(
                        out=r_sb[:, mi: mi + 1], in_=fold[:, :2 * w],
                        axis=mybir.AxisListType.X, op=mn)

            nc.sync.dma_start(out=a_dram.ap(), in_=a_sb[:])
            nc.sync.dma_start(out=r_dram.ap(), in_=r_sb[:])

    nc.compile()
    return nc


# ---------------------------------------------------------------------------
# Execution: a cached jitted shard_map runner over the 8 axon devices
# (rebuilding it per call would re-trace and cost ~0.5s/call), with a
# fallback to the stock run_bass_kernel_spmd path.
# ---------------------------------------------------------------------------

_RUNNER_CACHE = {}


def _make_runner(nc):
    import jax
    from jax.sharding import Mesh, PartitionSpec
    from jax.experimental.shard_map import shard_map
    from concourse import bass2jax
    from concourse.bass2jax import _bass_exec_p, install_neuronx_cc_hook

    install_neuronx_cc_hook()
    partition_name = nc.partition_id_tensor.name if nc.partition_id_tensor else None
    in_names, out_names, out_avals, zero_shapes = [], [], [], []
    for alloc in nc.m.functions[0].allocations:
        if not isinstance(alloc, mybir.MemoryLocationSet):
            continue
        name = alloc.memorylocations[0].name
        if alloc.kind == "ExternalInput":
            if name != partition_name:
                in_names.append(name)
        elif alloc.kind == "ExternalOutput":
            np_dtype = mybir.dt.np(alloc.dtype)
            shape = tuple(alloc.tensor_shape)
            out_names.append(name)
            out_avals.append(jax.core.ShapedArray(shape, np_dtype))
            zero_shapes.append((shape, np_dtype))

    n_params, n_outs = len(in_names), len(out_avals)
    all_in_names = list(in_names) + list(out_names)
    if partition_name is not None:
        all_in_names.append(partition_name)
    donate = tuple(range(n_params, n_params + n_outs))

    def _body(*args):
        operands = list(args)
        if partition_name is not None:
            operands.append(bass2jax.partition_id_tensor())
        outs = _bass_exec_p.bind(
            *operands, out_avals=tuple(out_avals),
            in_names=tuple(all_in_names), out_names=tuple(out_names),
            lowering_input_output_aliases=(),
            sim_require_finite=True, sim_require_nnan=True, nc=nc)
        return tuple(outs)

    devices = jax.devices()[:NCORES]
    mesh = Mesh(np.asarray(devices), ("core",))
    sharded = jax.jit(
        shard_map(_body, mesh=mesh,
                  in_specs=(PartitionSpec("core"),) * (n_params + n_outs),
                  out_specs=(PartitionSpec("core"),) * n_outs,
                  check_rep=False),
        donate_argnums=donate, keep_unused=True)

    def run(in_maps):
        concat_in = [
            np.concatenate([np.asarray(in_maps[c][name])
                            for c in range(NCORES)], axis=0)
            for name in in_names]
        zeros = [np.zeros((NCORES * s[0], *s[1:]), d) for s, d in zero_shapes]
        outs = sharded(*concat_in, *zeros)
        return [
            {name: np.asarray(outs[i]).reshape(NCORES, *out_avals[i].shape)[c]
             for i, name in enumerate(out_names)}
            for c in range(NCORES)]

    return run


def _run_spmd(nc, in_maps):
    key = id(nc)
    try:
        if key not in _RUNNER_CACHE:
            _RUNNER_CACHE[key] = _make_runner(nc)
        return _RUNNER_CACHE[key](in_maps)
    except Exception:
        from concourse.bass_utils import run_bass_kernel_spmd
        return run_bass_kernel_spmd(
            nc, in_maps, core_ids=list(range(NCORES))).results


def _host_reduce(results):
    """Combine per-core outputs into the final scalar (float64 internally)."""
    chamfers = []
    for b in range(B):
        fs = []
        bvecs = []
        for h in range(2):
            res = results[2 * b + h]
            A = np.asarray(res["a_out"]).astype(np.float64)   # [128, 4096]
            R = np.asarray(res["r_out"]).astype(np.float64)   # [128, 64|128]
            fs.append(A.min(axis=0))                          # [4096]
            if R.shape[1] == 2 * MI:                          # per-g columns
                R = np.minimum(R[:, :MI], R[:, MI:])          # [128, 64]
            bvecs.append(R.T.reshape(N))                      # m = 128*mi + p
        f = np.maximum(np.concatenate(fs), 0.0)               # [8192] fwd mins
        bv = np.maximum(np.minimum(bvecs[0], bvecs[1]), 0.0)  # [8192] bwd mins
        chamfers.append(f.mean() + bv.mean())
    return np.float32(np.mean(chamfers))


def kernel(pred, target):
    pred = np.asarray(pred, dtype=np.float32)
    target = np.asarray(target, dtype=np.float32)
    in_maps = _aug_inputs(pred, target)
    nc = _build_program()
    results = _run_spmd(nc, in_maps)
    return np.array(_host_reduce(results), dtype=np.float32)
